# revision 31
# baseline (speedup 1.0000x reference)
"""Trainium2 Bass kernel for the LSTM decoder — v6: four interleaved
quarter-batch recurrences, host-computed step 0.

Per core (256 batch rows) the batch is split into four independent 64-column
recurrences phase-shifted by a quarter step.  The Activation engine is the
busiest; with four streams its work arrives as (tanh(q), sigma(q+1)) pairs
of ~1.44us, one pair per quarter-phase, so ACT packs to ~90%+ and sets the
period, instead of the serial sigma -> c-chain -> tanh loop that limited a
two-half schedule.

- Step 0 (x = z) is computed on the host in fp32 (exact) and uploaded as
  tiny h8/h16/c state tiles; the device runs steps 1..31.  This removes the
  W_ih weights, the z upload, and the slow pipeline-fill step entirely.
- PSUM per quarter: one [128, 1024] region (2 banks): bank0 = [i, g] tiles,
  bank1 = [f, o] tiles, 16 tiles of [128 gate rows, 64 batch].
- One merged sigmoid ACT [128, 1024] per quarter-step covers all four gate
  classes (tanh(g) = 2*sigmoid(2g)-1 with the g-rows of W pre-doubled);
  one [128, 256] tanh for the c update.
- DVE per quarter-step: gtil = 2*u_g-1 (4x tensor_scalar), t2 = u_f*c,
  t1 = u_i*gtil, c' = t1+t2, h8b, h16; h8a rides on DVE too (Pool's latency
  is too high for the kg-gating chunk); h16 (for y) runs on Pool.
- fp8 (e4m3) DoubleRow matmuls: the two K-slots carry the (hi, lo) split of
  the merged W = W_ih + W_hh (valid since output h feeds back as the next
  input); moving operand is the fp8 h chunk broadcast into both slots.
- Emission per quarter-step: kg -> sigma/chain -> y_mm(t-1) -> y_copy ->
  bias(t+1): kg starts the moment h8 lands; y/bias run in the slack after
  sigma's PSUM read (y lands in the o-s0 PSUM tile, Pool copies it out).
"""

import numpy as np
import ml_dtypes
from contextlib import ExitStack

import concourse.bacc as bacc
import concourse.mybir as mybir
from concourse import tile
from concourse.bass_utils import run_bass_kernel_spmd

fp32 = mybir.dt.float32
fp16 = mybir.dt.float16
fp8 = mybir.dt.float8e4
F8 = ml_dtypes.float8_e4m3fn
AF = mybir.ActivationFunctionType
ALU = mybir.AluOpType
DR = mybir.MatmulPerfMode.DoubleRow

P = 128
B = 256          # batch rows per core
NQ = 4           # interleaved recurrences per core
QW = 64          # quarter-batch width
HC = 4           # hidden chunks of 128
NT = 16          # gate tiles per quarter
PH = 32
NCORES = 8
SP = 1024.0      # weight/bias scale (keeps all fp8 <= 240: IEEE-e4m3 safe)

# gate-class order in PSUM banks: [i, g | f, o]; W row bases (torch i,f,g,o)
RB = [0, 1024, 512, 1536]

_CACHE = {}


def _build():
    nc = bacc.Bacc("TRN2", target_bir_lowering=False, debug=False,
                   num_devices=NCORES)

    ws_d = nc.dram_tensor("ws", [P, HC, 2, NT, P], fp8, kind="ExternalInput")
    bs_d = nc.dram_tensor("bs", [1, 2, NT, P], fp8, kind="ExternalInput")
    wd_d = nc.dram_tensor("wd", [P, HC, 2], fp16, kind="ExternalInput")
    h80_d = nc.dram_tensor("h80", [P, NQ, HC * QW], fp8, kind="ExternalInput")
    h160_d = nc.dram_tensor("h160", [P, NQ, HC * QW], fp16,
                            kind="ExternalInput")
    c0_d = nc.dram_tensor("c0", [P, NQ, HC * QW], fp16, kind="ExternalInput")
    ones_d = nc.dram_tensor("ones", [1, QW], fp8, kind="ExternalInput")
    y_d = nc.dram_tensor("y", [2, PH * B], fp32, kind="ExternalOutput")

    with tile.TileContext(nc) as tc:
        with ExitStack() as ctx:
            const = ctx.enter_context(tc.tile_pool(name="const", bufs=1))
            state = ctx.enter_context(tc.tile_pool(name="state", bufs=1))
            pp = ctx.enter_context(tc.tile_pool(name="pp", bufs=1,
                                                space="PSUM"))

            pH = [pp.tile([P, 1024], fp32, tag=f"p{q}", name=f"p{q}")
                  for q in range(NQ)]
            u = [state.tile([P, 1024], fp16, tag=f"u{q}", name=f"u{q}")
                 for q in range(NQ)]
            ct = [state.tile([P, 256], fp16, tag=f"c{q}", name=f"c{q}")
                  for q in range(NQ)]
            tct = [state.tile([P, 256], fp16, tag=f"tc{q}", name=f"tc{q}")
                   for q in range(NQ)]
            gt = [state.tile([P, 256], fp16, tag=f"gt{q}", name=f"gt{q}")
                  for q in range(NQ)]
            t1 = [state.tile([P, 256], fp16, tag=f"t1{q}", name=f"t1{q}")
                  for q in range(NQ)]
            t2 = [state.tile([P, 256], fp16, tag=f"t2{q}", name=f"t2{q}")
                  for q in range(NQ)]
            h8t = [state.tile([P, NQ, HC * QW], fp8, tag=f"h8b{b}",
                              name=f"h8b{b}") for b in range(2)]
            h16t = [state.tile([P, NQ, HC * QW], fp16, tag=f"h16b{b}",
                               name=f"h16b{b}") for b in range(2)]
            h8 = [[h8t[b][:, q] for b in range(2)] for q in range(NQ)]
            h16 = [[h16t[b][:, q] for b in range(2)] for q in range(NQ)]
            ctt = state.tile([P, NQ, 256], fp16, tag="ct", name="ct")
            ct = [ctt[:, q] for q in range(NQ)]
            y_sb = const.tile([2, PH * B], fp32)

            # few, large DMAs (each dma_start costs ~625ns on the single
            # HWDGE descriptor generator), ordered by first use: the 2MB ws
            # transfer is bandwidth-bound (~5.8us) so everything the first
            # matmuls need goes in front of it
            ones = const.tile([1, QW], fp8)
            nc.sync.dma_start(ones[:], ones_d[:])
            bs = const.tile([1, 2, NT, P], fp8)
            nc.sync.dma_start(bs[:], bs_d[:])
            nc.sync.dma_start(h8t[0][:], h80_d[:])
            ws = const.tile([P, HC, 2, NT, P], fp8)
            nc.sync.dma_start(ws[:, 0:2], ws_d[:, 0:2])
            nc.sync.dma_start(ws[:, 2:4], ws_d[:, 2:4])
            nc.sync.dma_start(ctt[:], c0_d[:])
            nc.sync.dma_start(h16t[0][:], h160_d[:])
            wd = const.tile([P, HC, 2], fp16)
            nc.sync.dma_start(wd[:], wd_d[:])

            ones_b = ones[:].unsqueeze(1).broadcast_to([1, 2, QW])

            def mov(src_ap):
                return src_ap.unsqueeze(1).broadcast_to([P, 2, QW])

            def out_ap(q, tau):
                return pH[q][:, QW * tau:QW * (tau + 1)]

            def bias_mm(q, beta, s):
                tau = 4 * beta + s
                nc.tensor.matmul(out_ap(q, tau),
                                 bs[0:1, :, tau, :], ones_b,
                                 start=(tau in (0, 8)),
                                 stop=False, perf_mode=DR)

            def kg_mm(q, tau, kc, src_ap, stop):
                nc.tensor.matmul(out_ap(q, tau),
                                 ws[:, kc, :, tau, :], mov(src_ap),
                                 start=False, stop=stop, perf_mode=DR)

            def hsrc(q, t, kc):
                return h8[q][t % 2][:, kc * QW:(kc + 1) * QW]

            def bias_all(q):
                for beta in range(4):
                    for s in range(HC):
                        bias_mm(q, beta, s)

            def kg(q, t):
                # kc01 first (gated by the first h8 chunk), then kc23
                for kcp in ((0, 1), (2, 3)):
                    for kc in kcp:
                        for tau in range(NT):
                            kg_mm(q, tau, kc, hsrc(q, t - 1, kc),
                                  stop=(kc == 3 and tau in (7, 15)))

            def y_mm(q, t):
                # fp16 matmul from the fp16 h copy; lands in the o-s0 PSUM
                # tile after the merged sigmoid reads it
                out = pH[q][0:2, 768:768 + QW]
                for kc in range(HC):
                    nc.tensor.matmul(out, wd[:, kc, :],
                                     h16[q][t % 2][:, kc * QW:(kc + 1) * QW],
                                     start=(kc == 0), stop=(kc == 3))

            def y_copy(q, t):
                # GPSIMD cannot access PSUM (BIR verifier), so DVE copies
                nc.vector.tensor_copy(y_sb[:, B * t + QW * q:
                                           B * t + QW * (q + 1)],
                                      pH[q][0:2, 768:768 + QW])

            def chain(q, t):
                nc.scalar.activation(u[q][:], pH[q][:], AF.Sigmoid,
                                     scale=1.0 / SP)
                # DVE c-chain
                nc.vector.tensor_scalar(gt[q][:], u[q][:, 256:512], 2.0, 1.0,
                                        ALU.mult, ALU.subtract)
                nc.vector.tensor_mul(t2[q][:], u[q][:, 512:768], ct[q][:])
                nc.vector.tensor_mul(t1[q][:], u[q][:, 0:256], gt[q][:])
                nc.vector.tensor_add(ct[q][:], t1[q][:], t2[q][:])
                nc.scalar.activation(tct[q][:], ct[q][:], AF.Tanh)
                hb = h8[q][t % 2]
                nc.vector.tensor_mul(hb[:, 0:128], u[q][:, 768:896],
                                     tct[q][:, 0:128])
                nc.vector.tensor_mul(hb[:, 128:256], u[q][:, 896:1024],
                                     tct[q][:, 128:256])
                # h16 feeds y_mm next step: slack on the Pool engine
                nc.gpsimd.tensor_mul(h16[q][t % 2][:], u[q][:, 768:1024],
                                     tct[q][:])

            # start the accumulation groups for step 1
            for q in range(NQ):
                bias_all(q)

            # --- steady steps ---
            # per quarter: kg runs as soon as h8 lands (no PE prefix);
            # y/bias run in the slack after sigma's PSUM read
            for t in range(1, PH):
                for q in range(NQ):
                    kg(q, t)
                    chain(q, t)
                    y_mm(q, t - 1)
                    y_copy(q, t - 1)
                    if t < PH - 1:
                        bias_all(q)

            # --- drain the y tail ---
            for q in range(NQ):
                y_mm(q, PH - 1)
                y_copy(q, PH - 1)
            nc.sync.dma_start(y_d[:], y_sb[:])
    nc.compile()
    return nc


def _get_nc():
    if "nc" not in _CACHE:
        _CACHE["nc"] = _build()
    return _CACHE["nc"]


def _enc8(x):
    return np.asarray(F8(np.asarray(x, np.float32)))


def _sigmoid(x):
    return 1.0 / (1.0 + np.exp(-x))


def _prep_inputs(z, W_ih, W_hh, b_ih, b_hh, W_d):
    z2 = np.asarray(z, np.float32).reshape(2048, 512)
    W_ih = np.asarray(W_ih, np.float32)
    W_sum = W_ih + np.asarray(W_hh, np.float32)
    bias = (np.asarray(b_ih, np.float32) + np.asarray(b_hh, np.float32))

    # ---- step 0 on the host (exact fp32): x = z, h0 = c0 = 0 ----
    g0 = z2 @ W_ih.T + np.asarray(b_ih, np.float32) \
        + np.asarray(b_hh, np.float32)
    i0 = _sigmoid(g0[:, 0:512])
    g0g = np.tanh(g0[:, 1024:1536])
    o0 = _sigmoid(g0[:, 1536:2048])
    c0 = i0 * g0g                      # f-gate * c0 term is zero
    h0 = o0 * np.tanh(c0)              # [2048, 512]

    def fold_w(W):
        W2 = W * SP
        W2[1024:1536] *= 2.0
        return W2

    W2 = fold_w(W_sum)
    Bp = bias * SP
    Bp[1024:1536] *= 2.0

    # tile tau = 4*beta + s -> W rows RB[beta] + 128*s
    rows = np.empty((NT, P), np.int64)
    for beta in range(4):
        for s in range(HC):
            rows[4 * beta + s] = RB[beta] + 128 * s + np.arange(P)

    def to_ws(W2f):
        hi = _enc8(W2f).astype(np.float32)
        lo = _enc8(W2f - hi).astype(np.float32)

        def lay(Wq):
            a = Wq[rows]                                      # [16,128,512]
            a = a.reshape(NT, P, HC, P).transpose(3, 2, 0, 1)  # [p,kc,tau,m]
            return a
        out = np.stack([lay(hi), lay(lo)], axis=2)             # [p,kc,2,tau,m]
        return np.ascontiguousarray(_enc8(out))

    ws = to_ws(W2)

    bhi = _enc8(Bp).astype(np.float32)
    blo = _enc8(Bp - bhi).astype(np.float32)
    bs = np.stack([bhi[rows], blo[rows]], axis=0)
    bs = np.ascontiguousarray(_enc8(bs[None]))                 # [1,2,16,128]

    Wd2 = np.asarray(W_d, np.float32) * SP
    wd = np.ascontiguousarray(
        Wd2.T.reshape(HC, P, 2).transpose(1, 0, 2)).astype(np.float16)

    ones = _enc8(np.ones((1, QW), np.float32))

    in_maps = []
    for cix in range(NCORES):
        # state layout per quarter: [p, kc*QW + b] = value for hidden unit
        # (128*kc + p), batch row (64*q + b) of this core
        def lay_state(X):                                      # [256, 512]
            a = X[cix * B:(cix + 1) * B].T                     # [512, 256]
            a = a.reshape(HC, P, NQ, QW).transpose(1, 2, 0, 3)  # [p,q,kc,b]
            return np.ascontiguousarray(a.reshape(P, NQ, HC * QW))

        h0c = lay_state(h0)
        c0c = lay_state(c0)
        in_maps.append({
            "ws": ws, "bs": bs, "wd": wd,
            "h80": _enc8(h0c),
            "h160": h0c.astype(np.float16),
            "c0": c0c.astype(np.float16),
            "ones": ones,
        })
    return in_maps


def run(inputs, trace=False, **kw):
    nc = _get_nc()
    in_maps = _prep_inputs(inputs["z"], inputs["W_ih"], inputs["W_hh"],
                           inputs["b_ih"], inputs["b_hh"], inputs["W_d"])
    res = run_bass_kernel_spmd(nc, in_maps, core_ids=list(range(NCORES)),
                               trace=trace, **kw)
    b_d = np.asarray(inputs["b_d"], np.float32)
    outs = []
    for cix in range(NCORES):
        arr = res.results[cix]["y"] / SP                       # [2, PH*B]
        outs.append(arr.reshape(2, PH, B).transpose(2, 1, 0))
    y = np.concatenate(outs, axis=0) + b_d[None, None, :]
    return np.ascontiguousarray(y, dtype=np.float32), res


def kernel(**inputs):
    y, _ = run(inputs, trace=False)
    return y


# revision 33
# speedup vs baseline: 1.0091x; 1.0091x over previous
"""Trainium2 Bass kernel for the LSTM decoder — v6: four interleaved
quarter-batch recurrences, host-computed step 0.

Per core (256 batch rows) the batch is split into four independent 64-column
recurrences phase-shifted by a quarter step.  The Activation engine is the
busiest; with four streams its work arrives as (tanh(q), sigma(q+1)) pairs
of ~1.44us, one pair per quarter-phase, so ACT packs to ~90%+ and sets the
period, instead of the serial sigma -> c-chain -> tanh loop that limited a
two-half schedule.

- Step 0 (x = z) is computed on the host in fp32 (exact) and uploaded as
  tiny h8/h16/c state tiles; the device runs steps 1..31.  This removes the
  W_ih weights, the z upload, and the slow pipeline-fill step entirely.
- PSUM per quarter: one [128, 1024] region (2 banks): bank0 = [i, g] tiles,
  bank1 = [f, o] tiles, 16 tiles of [128 gate rows, 64 batch].
- One merged sigmoid ACT [128, 1024] per quarter-step covers all four gate
  classes (tanh(g) = 2*sigmoid(2g)-1 with the g-rows of W pre-doubled);
  one [128, 256] tanh for the c update.
- DVE per quarter-step: gtil = 2*u_g-1 (4x tensor_scalar), t2 = u_f*c,
  t1 = u_i*gtil, c' = t1+t2, h8b, h16; h8a rides on DVE too (Pool's latency
  is too high for the kg-gating chunk); h16 (for y) runs on Pool.
- fp8 (e4m3) DoubleRow matmuls: the two K-slots carry the (hi, lo) split of
  the merged W = W_ih + W_hh (valid since output h feeds back as the next
  input); moving operand is the fp8 h chunk broadcast into both slots.
- Emission per quarter-step: kg -> sigma/chain -> y_mm(t-1) -> y_copy ->
  bias(t+1): kg starts the moment h8 lands; y/bias run in the slack after
  sigma's PSUM read (y lands in the o-s0 PSUM tile, Pool copies it out).
"""

import numpy as np
import ml_dtypes
from contextlib import ExitStack

import concourse.bacc as bacc
import concourse.mybir as mybir
from concourse import tile
from concourse.bass_utils import run_bass_kernel_spmd

fp32 = mybir.dt.float32
fp16 = mybir.dt.float16
fp8 = mybir.dt.float8e4
F8 = ml_dtypes.float8_e4m3fn
AF = mybir.ActivationFunctionType
ALU = mybir.AluOpType
DR = mybir.MatmulPerfMode.DoubleRow

P = 128
B = 256          # batch rows per core
NQ = 4           # interleaved recurrences per core
QW = 64          # quarter-batch width
HC = 4           # hidden chunks of 128
NT = 16          # gate tiles per quarter
PH = 32
NCORES = 8
SP = 1024.0      # weight/bias scale (keeps all fp8 <= 240: IEEE-e4m3 safe)

# gate-class order in PSUM banks: [i, g | f, o]; W row bases (torch i,f,g,o)
RB = [0, 1024, 512, 1536]

_CACHE = {}


def _build():
    nc = bacc.Bacc("TRN2", target_bir_lowering=False, debug=False,
                   num_devices=NCORES)

    ws_d = nc.dram_tensor("ws", [P, HC, 2, NT, P], fp8, kind="ExternalInput")
    bs_d = nc.dram_tensor("bs", [1, 2, NT, P], fp8, kind="ExternalInput")
    wd_d = nc.dram_tensor("wd", [P, HC, 2], fp16, kind="ExternalInput")
    h80_d = nc.dram_tensor("h80", [P, NQ, HC * QW], fp8, kind="ExternalInput")
    h160_d = nc.dram_tensor("h160", [P, NQ, HC * QW], fp16,
                            kind="ExternalInput")
    c0_d = nc.dram_tensor("c0", [P, NQ, HC * QW], fp16, kind="ExternalInput")
    ones_d = nc.dram_tensor("ones", [1, QW], fp8, kind="ExternalInput")
    y_d = nc.dram_tensor("y", [2, PH * B], fp32, kind="ExternalOutput")

    with tile.TileContext(nc) as tc:
        with ExitStack() as ctx:
            const = ctx.enter_context(tc.tile_pool(name="const", bufs=1))
            state = ctx.enter_context(tc.tile_pool(name="state", bufs=1))
            pp = ctx.enter_context(tc.tile_pool(name="pp", bufs=1,
                                                space="PSUM"))

            pH = [pp.tile([P, 1024], fp32, tag=f"p{q}", name=f"p{q}")
                  for q in range(NQ)]
            u = [state.tile([P, 1024], fp16, tag=f"u{q}", name=f"u{q}")
                 for q in range(NQ)]
            ct = [state.tile([P, 256], fp16, tag=f"c{q}", name=f"c{q}")
                  for q in range(NQ)]
            tct = [state.tile([P, 256], fp16, tag=f"tc{q}", name=f"tc{q}")
                   for q in range(NQ)]
            gt = [state.tile([P, 256], fp16, tag=f"gt{q}", name=f"gt{q}")
                  for q in range(NQ)]
            t1 = [state.tile([P, 256], fp16, tag=f"t1{q}", name=f"t1{q}")
                  for q in range(NQ)]
            t2 = [state.tile([P, 256], fp16, tag=f"t2{q}", name=f"t2{q}")
                  for q in range(NQ)]
            h8t = [state.tile([P, NQ, HC * QW], fp8, tag=f"h8b{b}",
                              name=f"h8b{b}") for b in range(2)]
            h16t = [state.tile([P, NQ, HC * QW], fp16, tag=f"h16b{b}",
                               name=f"h16b{b}") for b in range(2)]
            h8 = [[h8t[b][:, q] for b in range(2)] for q in range(NQ)]
            h16 = [[h16t[b][:, q] for b in range(2)] for q in range(NQ)]
            ctt = state.tile([P, NQ, 256], fp16, tag="ct", name="ct")
            ct = [ctt[:, q] for q in range(NQ)]
            y_sb = const.tile([2, PH * B], fp32)

            # few, large DMAs (each dma_start costs ~625ns on the single
            # HWDGE descriptor generator), ordered by first use: the 2MB ws
            # transfer is bandwidth-bound (~5.8us) so everything the first
            # matmuls need goes in front of it
            ws = const.tile([P, HC, 2, NT, P], fp8)
            nc.sync.dma_start(ws[:, 0:2], ws_d[:, 0:2])
            ones = const.tile([1, QW], fp8)
            nc.sync.dma_start(ones[:], ones_d[:])
            bs = const.tile([1, 2, NT, P], fp8)
            nc.sync.dma_start(bs[:], bs_d[:])
            nc.sync.dma_start(h8t[0][:], h80_d[:])
            nc.sync.dma_start(ws[:, 2:4], ws_d[:, 2:4])
            nc.sync.dma_start(ctt[:], c0_d[:])
            nc.sync.dma_start(h16t[0][:], h160_d[:])
            wd = const.tile([P, HC, 2], fp16)
            nc.sync.dma_start(wd[:], wd_d[:])

            ones_b = ones[:].unsqueeze(1).broadcast_to([1, 2, QW])

            def mov(src_ap):
                return src_ap.unsqueeze(1).broadcast_to([P, 2, QW])

            def out_ap(q, tau):
                return pH[q][:, QW * tau:QW * (tau + 1)]

            def bias_mm(q, beta, s):
                tau = 4 * beta + s
                nc.tensor.matmul(out_ap(q, tau),
                                 bs[0:1, :, tau, :], ones_b,
                                 start=(tau in (0, 8)),
                                 stop=False, perf_mode=DR)

            def kg_mm(q, tau, kc, src_ap, stop):
                nc.tensor.matmul(out_ap(q, tau),
                                 ws[:, kc, :, tau, :], mov(src_ap),
                                 start=False, stop=stop, perf_mode=DR)

            def hsrc(q, t, kc):
                return h8[q][t % 2][:, kc * QW:(kc + 1) * QW]

            def bias_all(q):
                for beta in range(4):
                    for s in range(HC):
                        bias_mm(q, beta, s)

            def kg(q, t):
                # kc01 first (gated by the first h8 chunk), then kc23
                for kcp in ((0, 1), (2, 3)):
                    for kc in kcp:
                        for tau in range(NT):
                            kg_mm(q, tau, kc, hsrc(q, t - 1, kc),
                                  stop=(kc == 3 and tau in (7, 15)))

            def y_mm(q, t):
                # fp16 matmul from the fp16 h copy; lands in the o-s0 PSUM
                # tile after the merged sigmoid reads it
                out = pH[q][0:2, 768:768 + QW]
                for kc in range(HC):
                    nc.tensor.matmul(out, wd[:, kc, :],
                                     h16[q][t % 2][:, kc * QW:(kc + 1) * QW],
                                     start=(kc == 0), stop=(kc == 3))

            def y_copy(q, t):
                # GPSIMD cannot access PSUM (BIR verifier), so DVE copies
                nc.vector.tensor_copy(y_sb[:, B * t + QW * q:
                                           B * t + QW * (q + 1)],
                                      pH[q][0:2, 768:768 + QW])

            def chain(q, t):
                nc.scalar.activation(u[q][:], pH[q][:], AF.Sigmoid,
                                     scale=1.0 / SP)
                # DVE c-chain
                nc.vector.tensor_scalar(gt[q][:], u[q][:, 256:512], 2.0, 1.0,
                                        ALU.mult, ALU.subtract)
                nc.vector.tensor_mul(t2[q][:], u[q][:, 512:768], ct[q][:])
                nc.vector.tensor_mul(t1[q][:], u[q][:, 0:256], gt[q][:])
                nc.vector.tensor_add(ct[q][:], t1[q][:], t2[q][:])
                nc.scalar.activation(tct[q][:], ct[q][:], AF.Tanh)
                if t < PH - 1:
                    hb = h8[q][t % 2]
                    nc.vector.tensor_mul(hb[:, 0:128], u[q][:, 768:896],
                                         tct[q][:, 0:128])
                    nc.vector.tensor_mul(hb[:, 128:256], u[q][:, 896:1024],
                                         tct[q][:, 128:256])
                    # h16 feeds y_mm next step: slack on the Pool engine
                    nc.gpsimd.tensor_mul(h16[q][t % 2][:], u[q][:, 768:1024],
                                         tct[q][:])
                else:
                    # last step: no next matmuls; h16 on DVE for low latency
                    nc.vector.tensor_mul(h16[q][t % 2][:], u[q][:, 768:1024],
                                         tct[q][:])

            # start the accumulation groups for step 1
            for q in range(NQ):
                bias_all(q)

            # --- steady steps ---
            # per quarter: kg runs as soon as h8 lands (no PE prefix);
            # y/bias run in the slack after sigma's PSUM read
            for t in range(1, PH):
                for q in range(NQ):
                    kg(q, t)
                    chain(q, t)
                    y_mm(q, t - 1)
                    y_copy(q, t - 1)
                    if t < PH - 1:
                        bias_all(q)

            # --- drain the y tail ---
            for q in range(NQ):
                y_mm(q, PH - 1)
                y_copy(q, PH - 1)
            nc.sync.dma_start(y_d[:], y_sb[:])
    nc.compile()
    return nc


def _get_nc():
    if "nc" not in _CACHE:
        _CACHE["nc"] = _build()
    return _CACHE["nc"]


def _enc8(x):
    return np.asarray(F8(np.asarray(x, np.float32)))


def _sigmoid(x):
    return 1.0 / (1.0 + np.exp(-x))


def _prep_inputs(z, W_ih, W_hh, b_ih, b_hh, W_d):
    z2 = np.asarray(z, np.float32).reshape(2048, 512)
    W_ih = np.asarray(W_ih, np.float32)
    W_sum = W_ih + np.asarray(W_hh, np.float32)
    bias = (np.asarray(b_ih, np.float32) + np.asarray(b_hh, np.float32))

    # ---- step 0 on the host (exact fp32): x = z, h0 = c0 = 0 ----
    g0 = z2 @ W_ih.T + np.asarray(b_ih, np.float32) \
        + np.asarray(b_hh, np.float32)
    i0 = _sigmoid(g0[:, 0:512])
    g0g = np.tanh(g0[:, 1024:1536])
    o0 = _sigmoid(g0[:, 1536:2048])
    c0 = i0 * g0g                      # f-gate * c0 term is zero
    h0 = o0 * np.tanh(c0)              # [2048, 512]

    def fold_w(W):
        W2 = W * SP
        W2[1024:1536] *= 2.0
        return W2

    W2 = fold_w(W_sum)
    Bp = bias * SP
    Bp[1024:1536] *= 2.0

    # tile tau = 4*beta + s -> W rows RB[beta] + 128*s
    rows = np.empty((NT, P), np.int64)
    for beta in range(4):
        for s in range(HC):
            rows[4 * beta + s] = RB[beta] + 128 * s + np.arange(P)

    def to_ws(W2f):
        hi = _enc8(W2f).astype(np.float32)
        lo = _enc8(W2f - hi).astype(np.float32)

        def lay(Wq):
            a = Wq[rows]                                      # [16,128,512]
            a = a.reshape(NT, P, HC, P).transpose(3, 2, 0, 1)  # [p,kc,tau,m]
            return a
        out = np.stack([lay(hi), lay(lo)], axis=2)             # [p,kc,2,tau,m]
        return np.ascontiguousarray(_enc8(out))

    ws = to_ws(W2)

    bhi = _enc8(Bp).astype(np.float32)
    blo = _enc8(Bp - bhi).astype(np.float32)
    bs = np.stack([bhi[rows], blo[rows]], axis=0)
    bs = np.ascontiguousarray(_enc8(bs[None]))                 # [1,2,16,128]

    Wd2 = np.asarray(W_d, np.float32) * SP
    wd = np.ascontiguousarray(
        Wd2.T.reshape(HC, P, 2).transpose(1, 0, 2)).astype(np.float16)

    ones = _enc8(np.ones((1, QW), np.float32))

    in_maps = []
    for cix in range(NCORES):
        # state layout per quarter: [p, kc*QW + b] = value for hidden unit
        # (128*kc + p), batch row (64*q + b) of this core
        def lay_state(X):                                      # [256, 512]
            a = X[cix * B:(cix + 1) * B].T                     # [512, 256]
            a = a.reshape(HC, P, NQ, QW).transpose(1, 2, 0, 3)  # [p,q,kc,b]
            return np.ascontiguousarray(a.reshape(P, NQ, HC * QW))

        h0c = lay_state(h0)
        c0c = lay_state(c0)
        in_maps.append({
            "ws": ws, "bs": bs, "wd": wd,
            "h80": _enc8(h0c),
            "h160": h0c.astype(np.float16),
            "c0": c0c.astype(np.float16),
            "ones": ones,
        })
    return in_maps


def run(inputs, trace=False, **kw):
    nc = _get_nc()
    in_maps = _prep_inputs(inputs["z"], inputs["W_ih"], inputs["W_hh"],
                           inputs["b_ih"], inputs["b_hh"], inputs["W_d"])
    res = run_bass_kernel_spmd(nc, in_maps, core_ids=list(range(NCORES)),
                               trace=trace, **kw)
    b_d = np.asarray(inputs["b_d"], np.float32)
    outs = []
    for cix in range(NCORES):
        arr = res.results[cix]["y"] / SP                       # [2, PH*B]
        outs.append(arr.reshape(2, PH, B).transpose(2, 1, 0))
    y = np.concatenate(outs, axis=0) + b_d[None, None, :]
    return np.ascontiguousarray(y, dtype=np.float32), res


def kernel(**inputs):
    y, _ = run(inputs, trace=False)
    return y


# revision 35
# speedup vs baseline: 1.0111x; 1.0020x over previous
"""Trainium2 Bass kernel for the LSTM decoder — v6: four interleaved
quarter-batch recurrences, host-computed step 0.

Per core (256 batch rows) the batch is split into four independent 64-column
recurrences phase-shifted by a quarter step.  The Activation engine is the
busiest; with four streams its work arrives as (tanh(q), sigma(q+1)) pairs
of ~1.44us, one pair per quarter-phase, so ACT packs to ~90%+ and sets the
period, instead of the serial sigma -> c-chain -> tanh loop that limited a
two-half schedule.

- Step 0 (x = z) is computed on the host in fp32 (exact) and uploaded as
  tiny h8/h16/c state tiles; the device runs steps 1..31.  This removes the
  W_ih weights, the z upload, and the slow pipeline-fill step entirely.
- PSUM per quarter: one [128, 1024] region (2 banks): bank0 = [i, g] tiles,
  bank1 = [f, o] tiles, 16 tiles of [128 gate rows, 64 batch].
- One merged sigmoid ACT [128, 1024] per quarter-step covers all four gate
  classes (tanh(g) = 2*sigmoid(2g)-1 with the g-rows of W pre-doubled);
  one [128, 256] tanh for the c update.
- DVE per quarter-step: gtil = 2*u_g-1 (4x tensor_scalar), t2 = u_f*c,
  t1 = u_i*gtil, c' = t1+t2, h8b, h16; h8a rides on DVE too (Pool's latency
  is too high for the kg-gating chunk); h16 (for y) runs on Pool.
- fp8 (e4m3) DoubleRow matmuls: the two K-slots carry the (hi, lo) split of
  the merged W = W_ih + W_hh (valid since output h feeds back as the next
  input); moving operand is the fp8 h chunk broadcast into both slots.
- Emission per quarter-step: kg -> sigma/chain -> y_mm(t-1) -> y_copy ->
  bias(t+1): kg starts the moment h8 lands; y/bias run in the slack after
  sigma's PSUM read (y lands in the o-s0 PSUM tile, Pool copies it out).
"""

import numpy as np
import ml_dtypes
from contextlib import ExitStack

import concourse.bacc as bacc
import concourse.mybir as mybir
from concourse import tile
from concourse.bass_utils import run_bass_kernel_spmd

fp32 = mybir.dt.float32
fp16 = mybir.dt.float16
fp8 = mybir.dt.float8e4
F8 = ml_dtypes.float8_e4m3fn
AF = mybir.ActivationFunctionType
ALU = mybir.AluOpType
DR = mybir.MatmulPerfMode.DoubleRow

P = 128
B = 256          # batch rows per core
NQ = 4           # interleaved recurrences per core
QW = 64          # quarter-batch width
HC = 4           # hidden chunks of 128
NT = 16          # gate tiles per quarter
PH = 32
NCORES = 8
SP = 1024.0      # weight/bias scale (keeps all fp8 <= 240: IEEE-e4m3 safe)

# gate-class order in PSUM banks: [i, g | f, o]; W row bases (torch i,f,g,o)
RB = [0, 1024, 512, 1536]

_CACHE = {}


def _build():
    nc = bacc.Bacc("TRN2", target_bir_lowering=False, debug=False,
                   num_devices=NCORES)

    ws_d = nc.dram_tensor("ws", [P, HC, 2, NT, P], fp8, kind="ExternalInput")
    bs_d = nc.dram_tensor("bs", [1, 2, NT, P], fp8, kind="ExternalInput")
    wd_d = nc.dram_tensor("wd", [P, HC, 2], fp16, kind="ExternalInput")
    h80_d = nc.dram_tensor("h80", [P, NQ, HC * QW], fp8, kind="ExternalInput")
    h160_d = nc.dram_tensor("h160", [P, NQ, HC * QW], fp16,
                            kind="ExternalInput")
    c0_d = nc.dram_tensor("c0", [P, NQ, HC * QW], fp16, kind="ExternalInput")
    ones_d = nc.dram_tensor("ones", [1, QW], fp8, kind="ExternalInput")
    y_d = nc.dram_tensor("y", [2, PH * B], fp32, kind="ExternalOutput")

    with tile.TileContext(nc) as tc:
        with ExitStack() as ctx:
            const = ctx.enter_context(tc.tile_pool(name="const", bufs=1))
            state = ctx.enter_context(tc.tile_pool(name="state", bufs=1))
            pp = ctx.enter_context(tc.tile_pool(name="pp", bufs=1,
                                                space="PSUM"))

            pH = [pp.tile([P, 1024], fp32, tag=f"p{q}", name=f"p{q}")
                  for q in range(NQ)]
            u = [state.tile([P, 1024], fp16, tag=f"u{q}", name=f"u{q}")
                 for q in range(NQ)]
            ct = [state.tile([P, 256], fp16, tag=f"c{q}", name=f"c{q}")
                  for q in range(NQ)]
            tct = [state.tile([P, 256], fp16, tag=f"tc{q}", name=f"tc{q}")
                   for q in range(NQ)]
            gt = [state.tile([P, 256], fp16, tag=f"gt{q}", name=f"gt{q}")
                  for q in range(NQ)]
            t1 = [state.tile([P, 256], fp16, tag=f"t1{q}", name=f"t1{q}")
                  for q in range(NQ)]
            t2 = [state.tile([P, 256], fp16, tag=f"t2{q}", name=f"t2{q}")
                  for q in range(NQ)]
            h8t = [state.tile([P, NQ, HC * QW], fp8, tag=f"h8b{b}",
                              name=f"h8b{b}") for b in range(2)]
            h16t = [state.tile([P, NQ, HC * QW], fp16, tag=f"h16b{b}",
                               name=f"h16b{b}") for b in range(2)]
            h8 = [[h8t[b][:, q] for b in range(2)] for q in range(NQ)]
            h16 = [[h16t[b][:, q] for b in range(2)] for q in range(NQ)]
            ctt = state.tile([P, NQ, 256], fp16, tag="ct", name="ct")
            ct = [ctt[:, q] for q in range(NQ)]
            y_sb = const.tile([2, PH * B], fp32)

            # few, large DMAs (each dma_start costs ~625ns on the single
            # HWDGE descriptor generator), ordered by first use: the 2MB ws
            # transfer is bandwidth-bound (~5.8us) so everything the first
            # matmuls need goes in front of it
            ws = const.tile([P, HC, 2, NT, P], fp8)
            nc.sync.dma_start(ws[:, 0], ws_d[:, 0])
            ones = const.tile([1, QW], fp8)
            nc.sync.dma_start(ones[:], ones_d[:])
            bs = const.tile([1, 2, NT, P], fp8)
            nc.sync.dma_start(bs[:], bs_d[:])
            nc.sync.dma_start(h8t[0][:], h80_d[:])
            nc.sync.dma_start(ws[:, 1], ws_d[:, 1])
            nc.sync.dma_start(ws[:, 2], ws_d[:, 2])
            nc.sync.dma_start(ws[:, 3], ws_d[:, 3])
            nc.sync.dma_start(ctt[:], c0_d[:])
            nc.sync.dma_start(h16t[0][:], h160_d[:])
            wd = const.tile([P, HC, 2], fp16)
            nc.sync.dma_start(wd[:], wd_d[:])

            ones_b = ones[:].unsqueeze(1).broadcast_to([1, 2, QW])

            def mov(src_ap):
                return src_ap.unsqueeze(1).broadcast_to([P, 2, QW])

            def out_ap(q, tau):
                return pH[q][:, QW * tau:QW * (tau + 1)]

            def bias_mm(q, beta, s):
                tau = 4 * beta + s
                nc.tensor.matmul(out_ap(q, tau),
                                 bs[0:1, :, tau, :], ones_b,
                                 start=(tau in (0, 8)),
                                 stop=False, perf_mode=DR)

            def kg_mm(q, tau, kc, src_ap, stop):
                nc.tensor.matmul(out_ap(q, tau),
                                 ws[:, kc, :, tau, :], mov(src_ap),
                                 start=False, stop=stop, perf_mode=DR)

            def hsrc(q, t, kc):
                return h8[q][t % 2][:, kc * QW:(kc + 1) * QW]

            def bias_all(q):
                for beta in range(4):
                    for s in range(HC):
                        bias_mm(q, beta, s)

            def kg(q, t):
                # kc01 first (gated by the first h8 chunk), then kc23
                for kcp in ((0, 1), (2, 3)):
                    for kc in kcp:
                        for tau in range(NT):
                            kg_mm(q, tau, kc, hsrc(q, t - 1, kc),
                                  stop=(kc == 3 and tau in (7, 15)))

            def y_mm(q, t):
                # fp16 matmul from the fp16 h copy; lands in the o-s0 PSUM
                # tile after the merged sigmoid reads it
                out = pH[q][0:2, 768:768 + QW]
                for kc in range(HC):
                    nc.tensor.matmul(out, wd[:, kc, :],
                                     h16[q][t % 2][:, kc * QW:(kc + 1) * QW],
                                     start=(kc == 0), stop=(kc == 3))

            def y_copy(q, t):
                # GPSIMD cannot access PSUM (BIR verifier), so DVE copies
                nc.vector.tensor_copy(y_sb[:, B * t + QW * q:
                                           B * t + QW * (q + 1)],
                                      pH[q][0:2, 768:768 + QW])

            def chain(q, t):
                nc.scalar.activation(u[q][:], pH[q][:], AF.Sigmoid,
                                     scale=1.0 / SP)
                # DVE c-chain
                nc.vector.tensor_scalar(gt[q][:], u[q][:, 256:512], 2.0, 1.0,
                                        ALU.mult, ALU.subtract)
                nc.vector.tensor_mul(t2[q][:], u[q][:, 512:768], ct[q][:])
                nc.vector.tensor_mul(t1[q][:], u[q][:, 0:256], gt[q][:])
                nc.vector.tensor_add(ct[q][:], t1[q][:], t2[q][:])
                nc.scalar.activation(tct[q][:], ct[q][:], AF.Tanh)
                if t < PH - 1:
                    hb = h8[q][t % 2]
                    nc.vector.tensor_mul(hb[:, 0:128], u[q][:, 768:896],
                                         tct[q][:, 0:128])
                    nc.vector.tensor_mul(hb[:, 128:256], u[q][:, 896:1024],
                                         tct[q][:, 128:256])
                    # h16 feeds y_mm next step: slack on the Pool engine
                    nc.gpsimd.tensor_mul(h16[q][t % 2][:], u[q][:, 768:1024],
                                         tct[q][:])
                else:
                    # last step: no next matmuls; h16 on DVE for low latency
                    nc.vector.tensor_mul(h16[q][t % 2][:], u[q][:, 768:1024],
                                         tct[q][:])

            # --- steady steps ---
            # per quarter: kg runs as soon as h8 lands (no PE prefix);
            # y/bias run in the slack after sigma's PSUM read.
            # bias(q, 1) is emitted just before kg(q, 1) so the first
            # quarter's sigma isn't stuck behind all four bias groups.
            for t in range(1, PH):
                for q in range(NQ):
                    if t == 1:
                        bias_all(q)
                    kg(q, t)
                    chain(q, t)
                    y_mm(q, t - 1)
                    y_copy(q, t - 1)
                    if t < PH - 1:
                        bias_all(q)

            # bulk of y goes out while step 31 finishes
            nc.sync.dma_start(y_d[:, 0:B * (PH - 1)], y_sb[:, 0:B * (PH - 1)])

            # --- drain the y tail ---
            for q in range(NQ):
                y_mm(q, PH - 1)
                y_copy(q, PH - 1)
            nc.sync.dma_start(y_d[:, B * (PH - 1):], y_sb[:, B * (PH - 1):])
    nc.compile()
    return nc


def _get_nc():
    if "nc" not in _CACHE:
        _CACHE["nc"] = _build()
    return _CACHE["nc"]


def _enc8(x):
    return np.asarray(F8(np.asarray(x, np.float32)))


def _sigmoid(x):
    return 1.0 / (1.0 + np.exp(-x))


def _prep_inputs(z, W_ih, W_hh, b_ih, b_hh, W_d):
    z2 = np.asarray(z, np.float32).reshape(2048, 512)
    W_ih = np.asarray(W_ih, np.float32)
    W_sum = W_ih + np.asarray(W_hh, np.float32)
    bias = (np.asarray(b_ih, np.float32) + np.asarray(b_hh, np.float32))

    # ---- step 0 on the host (exact fp32): x = z, h0 = c0 = 0 ----
    g0 = z2 @ W_ih.T + np.asarray(b_ih, np.float32) \
        + np.asarray(b_hh, np.float32)
    i0 = _sigmoid(g0[:, 0:512])
    g0g = np.tanh(g0[:, 1024:1536])
    o0 = _sigmoid(g0[:, 1536:2048])
    c0 = i0 * g0g                      # f-gate * c0 term is zero
    h0 = o0 * np.tanh(c0)              # [2048, 512]

    def fold_w(W):
        W2 = W * SP
        W2[1024:1536] *= 2.0
        return W2

    W2 = fold_w(W_sum)
    Bp = bias * SP
    Bp[1024:1536] *= 2.0

    # tile tau = 4*beta + s -> W rows RB[beta] + 128*s
    rows = np.empty((NT, P), np.int64)
    for beta in range(4):
        for s in range(HC):
            rows[4 * beta + s] = RB[beta] + 128 * s + np.arange(P)

    def to_ws(W2f):
        hi = _enc8(W2f).astype(np.float32)
        lo = _enc8(W2f - hi).astype(np.float32)

        def lay(Wq):
            a = Wq[rows]                                      # [16,128,512]
            a = a.reshape(NT, P, HC, P).transpose(3, 2, 0, 1)  # [p,kc,tau,m]
            return a
        out = np.stack([lay(hi), lay(lo)], axis=2)             # [p,kc,2,tau,m]
        return np.ascontiguousarray(_enc8(out))

    ws = to_ws(W2)

    bhi = _enc8(Bp).astype(np.float32)
    blo = _enc8(Bp - bhi).astype(np.float32)
    bs = np.stack([bhi[rows], blo[rows]], axis=0)
    bs = np.ascontiguousarray(_enc8(bs[None]))                 # [1,2,16,128]

    Wd2 = np.asarray(W_d, np.float32) * SP
    wd = np.ascontiguousarray(
        Wd2.T.reshape(HC, P, 2).transpose(1, 0, 2)).astype(np.float16)

    ones = _enc8(np.ones((1, QW), np.float32))

    in_maps = []
    for cix in range(NCORES):
        # state layout per quarter: [p, kc*QW + b] = value for hidden unit
        # (128*kc + p), batch row (64*q + b) of this core
        def lay_state(X):                                      # [256, 512]
            a = X[cix * B:(cix + 1) * B].T                     # [512, 256]
            a = a.reshape(HC, P, NQ, QW).transpose(1, 2, 0, 3)  # [p,q,kc,b]
            return np.ascontiguousarray(a.reshape(P, NQ, HC * QW))

        h0c = lay_state(h0)
        c0c = lay_state(c0)
        in_maps.append({
            "ws": ws, "bs": bs, "wd": wd,
            "h80": _enc8(h0c),
            "h160": h0c.astype(np.float16),
            "c0": c0c.astype(np.float16),
            "ones": ones,
        })
    return in_maps


def run(inputs, trace=False, **kw):
    nc = _get_nc()
    in_maps = _prep_inputs(inputs["z"], inputs["W_ih"], inputs["W_hh"],
                           inputs["b_ih"], inputs["b_hh"], inputs["W_d"])
    res = run_bass_kernel_spmd(nc, in_maps, core_ids=list(range(NCORES)),
                               trace=trace, **kw)
    b_d = np.asarray(inputs["b_d"], np.float32)
    outs = []
    for cix in range(NCORES):
        arr = res.results[cix]["y"] / SP                       # [2, PH*B]
        outs.append(arr.reshape(2, PH, B).transpose(2, 1, 0))
    y = np.concatenate(outs, axis=0) + b_d[None, None, :]
    return np.ascontiguousarray(y, dtype=np.float32), res


def kernel(**inputs):
    y, _ = run(inputs, trace=False)
    return y
